# revision 27
# baseline (speedup 1.0000x reference)
"""Trainium2 Bass kernel for a dense transformer block (pre-LN MHA + FFN).

Reference computation (B=2, S=2048, E=768, H=12, D=64, FF=3072):
    res = x
    xn = LN(x, gamma, beta)
    q,k,v = xn @ wq.T, xn @ wk.T, xn @ wv.T          (per-head D=64)
    attn = causal_softmax(q k^T / sqrt(D)) v
    res = res + attn @ wo.T + bo
    y = LN(res, gamma, beta)
    out = res + gelu(y @ w1.T + b1) @ w2.T + b2

Sharding: 8 cores, token-parallel with STRIDED assignment for causal load
balance. Core c (group g = c%4, batch b = c//4) owns tokens g::4 of its
batch, so its local q-tile n (128 tokens) covers sequence span
[512n, 512n+512) and needs exactly the key prefix [0, 512(n+1)) — the
same structure on every core. K^T and V are AllGather'ed within each
batch's 4-core group; attention computes only the causal prefix blocks.
The diagonal band (one 128-col region per key block) is masked by
ACCUMULATING -400 into the score PSUM via a matmul with the identity as
stationary (mask values are a per-core input), so exp sees masked scores
with no extra vector-engine op on the critical path.

Layout notes:
  - activations are token-major [tokens(part), feat(free)]; matmuls use
    PE-transposed activation tiles as the stationary operand and
    host-pre-transposed weights as the moving operand.
  - q/k are produced feature-major ([64, T] per head); scores come out
    [keys, q] (keys on partitions); the softmax denominator comes from an
    all-ones column appended to V ("V_aug"); normalization multiplies the
    feature-major attention output via a rank-1 broadcast matmul.
  - LN transposes batch 4 token-tiles into one PSUM bank, then one
    vector copy moves [128, 512] to SBUF.
"""

import os

import numpy as np

import concourse.bass as bass
import concourse.mybir as mybir
import concourse.tile as tile
from concourse import bacc
from concourse.bass_utils import run_bass_kernel_spmd
from concourse.masks import make_identity

F32 = mybir.dt.float32
BF16 = mybir.dt.bfloat16
AF = mybir.ActivationFunctionType

B, S, E, H, FF = 2, 2048, 768, 12, 3072
D = E // H                      # 64
NCORES = 8
T = B * S // NCORES             # 512 tokens per core
TN = T // 128                   # 4 token tiles per core
EK = E // 128                   # 6 feature chunks
FK = FF // 128                  # 24 hidden chunks
GROUP = NCORES // B             # 4 cores per batch
SB = S // 128                   # 16 key blocks per batch
EPS = 1e-5
SCALE = 1.0 / np.sqrt(D)
NEG = -400.0                    # additive mask (pre-scale); exp(-50) ~ 2e-22

DT_A = BF16                     # attention operand dtype
DT_P = BF16                     # FFN operand dtype
DT_X = mybir.dt.float8e4        # QKV/out-proj operand dtype (DoubleRow)
DR = mybir.MatmulPerfMode.DoubleRow
DEBUG_OUTPUTS = bool(int(os.environ.get("KDEBUG", "0")))
KPHASES = os.environ.get("KPHASES", "full")


def _ln(nc, pools, x_ap, out_ap, eps_sb):
    """Normalize (x - mu) * rstd over the free axis (768) of a [128, 768]
    tile. Heavy lifting on the Scalar engine: Square+accum_out gives sum(x^2),
    DVE reduce_sum gives sum(x); normalize is one ACT Identity with
    per-partition scale/bias. gamma/beta are applied downstream."""
    sp = pools["stats"]
    trash = sp.tile([128, E], BF16, tag="trash")
    ss = sp.tile([128, 1], F32, tag="ss")
    nc.scalar.activation(out=trash, in_=x_ap, func=AF.Square, accum_out=ss)
    sm = sp.tile([128, 1], F32, tag="sm")
    nc.vector.reduce_sum(out=sm, in_=x_ap, axis=mybir.AxisListType.X)
    mu = sp.tile([128, 1], F32, tag="mu")
    nc.vector.tensor_scalar_mul(mu, sm, 1.0 / E)
    mu2 = sp.tile([128, 1], F32, tag="mu2")
    nc.vector.tensor_mul(mu2, mu, mu)
    var = sp.tile([128, 1], F32, tag="var")
    # var = ss/E - mu^2
    nc.vector.tensor_scalar(out=var, in0=ss, scalar1=1.0 / E, scalar2=None,
                            op0=mybir.AluOpType.mult)
    nc.vector.tensor_sub(var, var, mu2)
    rstd = sp.tile([128, 1], F32, tag="rstd")
    nc.scalar.activation(out=rstd, in_=var, func=AF.Sqrt, bias=eps_sb, scale=1.0)
    nc.vector.reciprocal(out=rstd, in_=rstd)
    nbias = sp.tile([128, 1], F32, tag="nbias")
    nc.vector.tensor_tensor(out=nbias, in0=mu, in1=rstd, op=mybir.AluOpType.mult)
    nc.vector.tensor_scalar_mul(nbias, nbias, -1.0)
    nc.scalar.activation(out=out_ap, in_=x_ap, func=AF.Identity,
                         bias=nbias, scale=rstd)


def declare_io(nc):
    io = {}
    io["x_own"] = nc.dram_tensor("x_own", [T, E], F32, kind="ExternalInput").ap()
    for nm in ("wqT", "wkT", "wvT", "woT"):
        io[nm] = nc.dram_tensor(nm, [E, E], DT_X, kind="ExternalInput").ap()
    io["w1T"] = nc.dram_tensor("w1T", [E, FF], DT_P, kind="ExternalInput").ap()
    io["w2T"] = nc.dram_tensor("w2T", [FF, E], DT_P, kind="ExternalInput").ap()
    io["b1rs"] = nc.dram_tensor("b1rs", [128, FK], F32, kind="ExternalInput").ap()
    io["bo_row"] = nc.dram_tensor("bo_row", [1, E], F32, kind="ExternalInput").ap()
    io["b2_row"] = nc.dram_tensor("b2_row", [1, E], F32, kind="ExternalInput").ap()
    io["g_col"] = nc.dram_tensor("g_col", [128, EK], F32, kind="ExternalInput").ap()
    io["be_col"] = nc.dram_tensor("be_col", [128, EK], F32, kind="ExternalInput").ap()
    io["masks"] = nc.dram_tensor("masks", [4, 128, 128], DT_A, kind="ExternalInput").ap()
    io["out"] = nc.dram_tensor("out", [T, E], F32, kind="ExternalOutput").ap()
    if DEBUG_OUTPUTS:
        for nm, shp, dt in (("dbg_xnT", [128, EK * T], DT_P),
                            ("dbg_qT", [128, EK * T], DT_A),
                            ("dbg_attnT", [128, EK * T], DT_P),
                            ("dbg_res", [128, TN * E], F32)):
            io[nm] = nc.dram_tensor(nm, shp, dt, kind="ExternalOutput").ap()
    return io


def build_kernel_body(tc, io, skip_collectives=False):
    nc = tc.nc
    x_own, wqT, wkT, wvT, woT = (io[k] for k in ("x_own", "wqT", "wkT", "wvT", "woT"))
    w1T, w2T, b1rs = io["w1T"], io["w2T"], io["b1rs"]
    bo_row, b2_row, g_col, be_col = (io[k] for k in ("bo_row", "b2_row", "g_col", "be_col"))
    masks, out = io["masks"], io["out"]

    pools = {}
    ctx_pools = []

    def open_pool(name, **kw):
        cm = tc.tile_pool(name=name, **kw)
        pool = cm.__enter__()
        ctx_pools.append(cm)
        pool._cm = cm
        return pool

    persist = open_pool("persist", bufs=1)
    pools["stats"] = open_pool("stats", bufs=3)
    dram = open_pool("dram", bufs=1, space="DRAM")

    # ---- constants ----
    identity = persist.tile([128, 128], DT_P)
    make_identity(nc, identity)

    ones_all = persist.tile([128, 128], F32)
    nc.vector.memset(ones_all, 1.0)

    eps_sb = persist.tile([128, 1], F32)
    nc.vector.memset(eps_sb, EPS)

    def rep128(name, row_ap):
        t = persist.tile([128, E], F32, name=name)
        src = bass.AP(tensor=row_ap.tensor, offset=row_ap.offset,
                      ap=[[0, 128]] + list(row_ap.ap[1:]))
        nc.sync.dma_start(out=t, in_=src)
        return t

    bo_rep = rep128("bo_rep", bo_row)
    b2_rep = rep128("b2_rep", b2_row)
    b1_sb = persist.tile([128, FK], F32)
    nc.sync.dma_start(out=b1_sb, in_=b1rs)
    g_sb = persist.tile([128, EK], F32)
    nc.sync.dma_start(out=g_sb, in_=g_col)
    be_sb = persist.tile([128, EK], F32)
    nc.sync.dma_start(out=be_sb, in_=be_col)
    mask_sb = persist.tile([128, 4, 128], DT_A)
    nc.sync.dma_start(out=mask_sb, in_=masks.rearrange("t p q -> p t q"))

    # ---- long-lived activations ----
    qT_sb = persist.tile([128, EK, T], DT_A)      # q, feature-major
    attnT_sb = persist.tile([128, EK, T], DT_X)   # attention out, feature-major
    res_sb = persist.tile([128, TN, E], F32)      # post-attn residual, token-major

    # ---- AllGather bounce buffers ----
    VC = H * (D + 1)            # 780: V columns with interleaved ones
    ag_k_in = dram.tile([E, T], DT_A)
    ag_v_in = dram.tile([T, VC], DT_A)
    ag_k_out = dram.tile([GROUP * E, T], DT_A)
    ag_v_out = dram.tile([S, VC], DT_A)
    groups = [list(range(GROUP)), list(range(GROUP, NCORES))]

    # ================= Phase A/B: LN1, x^T, QKV =================
    with tc.tile_pool(name="span_ab", bufs=1) as pab, \
         tc.tile_pool(name="tr_ab", bufs=2) as ptr, \
         tc.tile_pool(name="wproj", bufs=2) as pw, \
         tc.tile_pool(name="psum_ta", bufs=1, space="PSUM") as ppta, \
         tc.tile_pool(name="psum_ab", bufs=3, space="PSUM") as pps:
        x_sb = pab.tile([128, TN, E], F32)
        x_sb_keep = x_sb
        xr = x_own.rearrange("(n p) e -> p n e", p=128)
        for n in range(TN):
            nc.sync.dma_start(out=x_sb[:, n, :], in_=xr[:, n, :])
        xn_b = pab.tile([128, TN, E], DT_P)
        for n in range(TN):
            _ln(nc, pools, x_sb[:, n, :], xn_b[:, n, :], eps_sb)
        xnT_sb = pab.tile([128, EK, T], DT_X)
        for e in range(EK):
            tp = ppta.tile([128, T], DT_P, tag="tp")
            for n in range(TN):
                nc.tensor.transpose(tp[:, n * 128:(n + 1) * 128],
                                    xn_b[:, n, e * 128:(e + 1) * 128], identity)
            nc.vector.tensor_scalar(
                out=xnT_sb[:, e, :], in0=tp, scalar1=g_sb[:, e:e + 1],
                scalar2=be_sb[:, e:e + 1],
                op0=mybir.AluOpType.mult, op1=mybir.AluOpType.add)

        # k^T first (its AllGather gates nothing downstream of qT), then V
        # (its AllGather gates attention head 0), then local q^T last.
        wk_sb = pw.tile([128, EK, E], DT_X, tag="w")
        wkr = wkT.rearrange("(k p) f -> p k f", p=128)
        for k in range(EK):
            nc.sync.dma_start(out=wk_sb[:, k, :], in_=wkr[:, k, :])
        for m in range(EK):
            ps = pps.tile([128, T], F32, tag="mm")
            for c in range(EK // 2):
                nc.tensor.matmul(ps, wk_sb[:, 2 * c:2 * c + 2, m * 128:(m + 1) * 128],
                                 xnT_sb[:, 2 * c:2 * c + 2, :],
                                 start=(c == 0), stop=(c == EK // 2 - 1), perf_mode=DR)
            kcp = ptr.tile([128, T], DT_A, tag="kcp")
            nc.vector.tensor_copy(kcp, ps)
            nc.sync.dma_start(out=ag_k_in[m * 128:(m + 1) * 128, :], in_=kcp)

        # V: token-major [T, 768]
        wv_sb = pw.tile([128, EK, E], DT_X, tag="w")
        wvr = wvT.rearrange("(k p) f -> p k f", p=128)
        for k in range(EK):
            nc.sync.dma_start(out=wv_sb[:, k, :], in_=wvr[:, k, :])
        for n in range(TN):
            psv = pps.tile([128, E], F32, tag="mmv", bufs=2)
            for c in range(EK // 2):
                lhsT = xnT_sb[:, 2 * c:2 * c + 2, n * 128:(n + 1) * 128]
                nc.tensor.matmul(psv[:, 0:512], lhsT, wv_sb[:, 2 * c:2 * c + 2, 0:512],
                                 start=(c == 0), stop=(c == EK // 2 - 1), perf_mode=DR)
                nc.tensor.matmul(psv[:, 512:768], lhsT,
                                 wv_sb[:, 2 * c:2 * c + 2, 512:768],
                                 start=(c == 0), stop=(c == EK // 2 - 1), perf_mode=DR)
            vcp = ptr.tile([128, VC], DT_A, tag="vcp")
            vcpr = vcp.rearrange("p (h c) -> p h c", h=H, c=D + 1)
            nc.vector.tensor_copy(vcpr[:, :, 0:D],
                                  psv.rearrange("p (h c) -> p h c", h=H, c=D))
            nc.vector.memset(vcpr[:, :, D:D + 1], 1.0)
            nc.sync.dma_start(
                out=ag_v_in.rearrange("(n p) c -> p n c", p=128)[:, n, :], in_=vcp)

        # q^T: local only, overlaps the in-flight AllGathers
        wq_sb = pw.tile([128, EK, E], DT_X, tag="w")
        wqr = wqT.rearrange("(k p) f -> p k f", p=128)
        for k in range(EK):
            nc.sync.dma_start(out=wq_sb[:, k, :], in_=wqr[:, k, :])
        for m in range(EK):
            ps = pps.tile([128, T], F32, tag="mm")
            for c in range(EK // 2):
                nc.tensor.matmul(ps, wq_sb[:, 2 * c:2 * c + 2, m * 128:(m + 1) * 128],
                                 xnT_sb[:, 2 * c:2 * c + 2, :],
                                 start=(c == 0), stop=(c == EK // 2 - 1), perf_mode=DR)
            nc.vector.tensor_copy(qT_sb[:, m, :], ps)


    if DEBUG_OUTPUTS:
        nc.sync.dma_start(out=io["dbg_xnT"],
                          in_=xnT_sb.rearrange("p k t -> p (k t)"))
        nc.sync.dma_start(out=io["dbg_qT"], in_=qT_sb.rearrange("p k t -> p (k t)"))

    # ================= Phase C: AllGather K^T and V =================
    if not skip_collectives:
        nc.gpsimd.collective_compute("AllGather", mybir.AluOpType.bypass,
                                     replica_groups=groups,
                                     ins=[ag_k_in[:]], outs=[ag_k_out[:]])
        nc.gpsimd.collective_compute("AllGather", mybir.AluOpType.bypass,
                                     replica_groups=groups,
                                     ins=[ag_v_in[:]], outs=[ag_v_out[:]])

    # ---- open late-phase pools early so weight DMAs overlap attention ----
    span_fgh = open_pool("span_fgh", bufs=1)
    hT_sb = span_fgh.tile([128, FK, T], DT_P)     # FFN hidden, feature-major
    wo_sb = span_fgh.tile([128, EK, E], DT_X)
    wor = woT.rearrange("(k p) f -> p k f", p=128)
    for k in range(EK):
        nc.gpsimd.dma_start(out=wo_sb[:, k, :], in_=wor[:, k, :])
    w1_sb = span_fgh.tile([128, EK, FF], DT_P)
    w1r = w1T.rearrange("(k p) f -> p k f", p=128)
    for k in range(EK):
        for j in range(2):
            nc.gpsimd.dma_start(out=w1_sb[:, k, j * 1536:(j + 1) * 1536],
                                in_=w1r[:, k, j * 1536:(j + 1) * 1536])
    w2_sb = span_fgh.tile([128, FK, E], DT_P)
    w2r = w2T.rearrange("(k p) f -> p k f", p=128)
    for k in range(0, FK, 2):
        nc.gpsimd.dma_start(out=w2_sb[:, k:k + 2, :], in_=w2r[:, k:k + 2, :])
    x_rl = span_fgh.tile([128, TN, E], F32)
    xr2 = x_own.rearrange("(n p) e -> p n e", p=128)
    for n in range(TN):
        nc.gpsimd.dma_start(out=x_rl[:, n, :], in_=xr2[:, n, :])
    for n in range(TN):
        nc.gpsimd.tensor_add(x_rl[:, n, :], x_rl[:, n, :], bo_rep)

    if KPHASES == "ab":
        outr0 = out.rearrange("(n p) e -> p n e", p=128)
        nc.sync.dma_start(out=outr0[:, 0, :], in_=x_sb_keep[:, 0, :])
        for p in reversed(ctx_pools):
            p.__exit__(None, None, None)
        return

    # ================= Phase D/E: attention =================
    # Per head: 8 pairs of key blocks, pair pi=(m,p) holds blocks t=4m+2p+j
    # (j=0,1), each computing scores for the q suffix [128m, 512). The local
    # first 128 columns of each block's region are its diagonal band: add
    # NEG there via matmul (identity stationary, mask moving), then exp.
    run_attn = KPHASES != "abfgh"
    with tc.tile_pool(name="attn_v", bufs=1) as pvb, \
         tc.tile_pool(name="attn_kv", bufs=3) as pkv, \
         tc.tile_pool(name="attn_exp", bufs=4) as pexp, \
         tc.tile_pool(name="attn_nrm", bufs=2) as pnrm, \
         tc.tile_pool(name="psum_s", bufs=3, space="PSUM") as pps_s, \
         tc.tile_pool(name="psum_a", bufs=2, space="PSUM") as pps_a:
        kgr = ag_k_out.rearrange("(r hp p) t -> p r hp t", r=GROUP, hp=H // 2, p=2 * D)
        vgr = ag_v_out.rearrange("(r n p) c -> r p n c", r=GROUP, n=SB, p=32)
        vbig = pvb.tile([128, SB, VC], DT_A)
        for r in range(GROUP):
            nc.sync.dma_start(out=vbig[r * 32:(r + 1) * 32, :, :], in_=vgr[r])
        if not run_attn:
            nc.vector.memset(attnT_sb[:, 0, 0:1], 0.0)

        def emit_qk(st):
            ps_pair, m, p, h = st["ps"], st["m"], st["p"], st["h"]
            khead = st["khead"]
            base = (h % 2) * D
            L = T - 128 * m
            qs = 128 * m
            for j in range(2):
                t = 4 * m + 2 * p + j
                kap = khead[base:base + D, t, :]          # 128 keys, contiguous
                nc.tensor.matmul(ps_pair[:, j, 0:L], kap,
                                 qT_sb[base:base + D, h // 2, qs:T],
                                 start=True, stop=False)
            for j in range(2):
                i = 2 * p + j
                nc.tensor.matmul(ps_pair[:, j, 0:128], identity, mask_sb[:, i, :],
                                 start=False, stop=True)

        def emit_exp(st):
            ps_pair, m = st["ps"], st["m"]
            L = T - 128 * m
            ex = pexp.tile([128, 2, T], DT_A, tag="ex")
            st["ex"] = ex
            nc.scalar.activation(out=ex[:, :, 0:L], in_=ps_pair[:, :, 0:L],
                                 func=AF.Exp, scale=SCALE)

        def emit_pv(st):
            ex, ps_attn, m, p, h = st["ex"], st["ps_attn"], st["m"], st["p"], st["h"]
            L = T - 128 * m
            qs = 128 * m
            for j in range(2):
                t = 4 * m + 2 * p + j
                nc.tensor.matmul(ps_attn[0:D + 1, qs:T],
                                 vbig[:, t, h * (D + 1):(h + 1) * (D + 1)],
                                 ex[:, j, 0:L], start=(t == 0), stop=(t == SB - 1))

        def emit_norm(hst):
            ps_attn, h = hst["ps_attn"], hst["h"]
            att = pnrm.tile([D + 1, T], F32, tag="att")
            nc.vector.tensor_copy(att, ps_attn[0:D + 1, :])
            nc.vector.reciprocal(att[D:D + 1, :], att[D:D + 1, :])
            ps_r = pps_s.tile([128, 2, T], F32, tag="ps")
            nc.tensor.matmul(ps_r[0:D, 0, :], ones_all[D:D + 1, 0:D],
                             att[D:D + 1, :], start=True, stop=True)
            nc.vector.tensor_mul(
                attnT_sb[(h % 2) * D:(h % 2 + 1) * D, h // 2, :],
                att[0:D, :], ps_r[0:D, 0, :])

        # flat software pipeline across all heads: at step G emit
        # QK(G), exp(G-1), PV(G-2); norm(h) rides right after PV(h, 7)
        stages = [{"h": h, "m": pi // 2, "p": pi % 2}
                  for h in range(H) for pi in range(8)]
        if not run_attn:
            stages = []
        heads = {}
        khead = None
        nst = len(stages)
        for G in range(nst + 2):
            if G < nst:
                st = stages[G]
                h = st["h"]
                if st["m"] == 0 and st["p"] == 0:
                    if h % 2 == 0:
                        # khead[:, t, r*32+jj] = K^T col (token r + 4*(32t+jj));
                        # key blocks contiguous per t (stationary needs 1D free)
                        khead = pkv.tile([128, SB, 128], DT_A, tag="khead")
                        for r in range(GROUP):
                            nc.sync.dma_start(
                                out=khead[:, :, r * 32:(r + 1) * 32],
                                in_=kgr[:, r, h // 2, :].rearrange(
                                    "p (t j) -> p t j", j=32))
                    ps_attn = pps_a.tile([D + 1, T], F32, tag="pattn")
                    heads[h] = {"h": h, "ps_attn": ps_attn, "khead": khead}
                st["khead"] = heads[h]["khead"]
                st["ps_attn"] = heads[h]["ps_attn"]
                st["ps"] = pps_s.tile([128, 2, T], F32, tag="ps", name="ps_pair")
                emit_qk(st)
            if 1 <= G < nst + 1:
                emit_exp(stages[G - 1])
            if 2 <= G < nst + 2:
                s2 = stages[G - 2]
                emit_pv(s2)
                if s2["m"] == 3 and s2["p"] == 1:
                    emit_norm(heads[s2["h"]])

    if DEBUG_OUTPUTS:
        nc.sync.dma_start(out=io["dbg_attnT"],
                          in_=attnT_sb.rearrange("p k t -> p (k t)"))

    # ================= Phases F-H =================
    if KPHASES == "abd":
        outr0 = out.rearrange("(n p) e -> p n e", p=128)
        nc.sync.dma_start(out=outr0[:, 0, :], in_=x_sb_keep[:, 0, :])
        for p in reversed(ctx_pools):
            p.__exit__(None, None, None)
        return

    span_fg = open_pool("span_fg", bufs=1)
    y2n_b = span_fg.tile([128, TN, E], DT_P)
    y2nT_sb = span_fg.tile([128, EK, T], DT_P)    # LN2 output, feature-major

    # ---- Phase F: out-proj + residual + LN2 ----
    with tc.tile_pool(name="psum_tf", bufs=1, space="PSUM") as pptf, \
         tc.tile_pool(name="psum_f", bufs=2, space="PSUM") as ppsf:
        tpf = pptf.tile([128, EK, T], DT_P)
        for n in range(TN):
            psf = ppsf.tile([128, E], F32, tag="mm")
            for c in range(EK // 2):
                lhsT = attnT_sb[:, 2 * c:2 * c + 2, n * 128:(n + 1) * 128]
                nc.tensor.matmul(psf[:, 0:512], lhsT, wo_sb[:, 2 * c:2 * c + 2, 0:512],
                                 start=(c == 0), stop=(c == EK // 2 - 1), perf_mode=DR)
                nc.tensor.matmul(psf[:, 512:768], lhsT,
                                 wo_sb[:, 2 * c:2 * c + 2, 512:768],
                                 start=(c == 0), stop=(c == EK // 2 - 1), perf_mode=DR)
            nc.vector.tensor_add(res_sb[:, n, :], psf, x_rl[:, n, :])
            _ln(nc, pools, res_sb[:, n, :], y2n_b[:, n, :], eps_sb)
            for e in range(EK):
                nc.tensor.transpose(tpf[:, e, n * 128:(n + 1) * 128],
                                    y2n_b[:, n, e * 128:(e + 1) * 128], identity)
        for e in range(EK):
            nc.vector.tensor_copy(y2nT_sb[:, e, :], tpf[:, e, :])

    if DEBUG_OUTPUTS:
        nc.sync.dma_start(out=io["dbg_res"],
                          in_=res_sb.rearrange("p n e -> p (n e)"))

    # ================= Phase G: FFN1 + GELU -> hT =================
    with tc.tile_pool(name="psum_g", bufs=4, space="PSUM") as ppsg:
        for m in range(FK):
            ps = ppsg.tile([128, T], F32, tag="mm")
            for k in range(EK):
                nc.tensor.matmul(ps, w1_sb[:, k, m * 128:(m + 1) * 128],
                                 y2nT_sb[:, k, :], start=(k == 0), stop=(k == EK - 1))
            nc.scalar.activation(out=hT_sb[:, m, :], in_=ps, func=AF.Gelu,
                                 bias=b1_sb[:, m:m + 1], scale=1.0)

    # close span_fg (y2nT dead) before loading anything else
    ctx_pools.remove(span_fg._cm)
    span_fg._cm.__exit__(None, None, None)

    # ================= Phase H: FFN2 + residual -> out =================
    with tc.tile_pool(name="psum_h", bufs=3, space="PSUM") as ppsh:
        for n in range(TN):
            ps = ppsh.tile([128, E], F32, tag="mm")
            for k in range(FK):
                lhsT = hT_sb[:, k, n * 128:(n + 1) * 128]
                nc.tensor.matmul(ps[:, 0:512], lhsT, w2_sb[:, k, 0:512],
                                 start=(k == 0), stop=(k == FK - 1))
                nc.tensor.matmul(ps[:, 512:768], lhsT, w2_sb[:, k, 512:768],
                                 start=(k == 0), stop=(k == FK - 1))
            nc.vector.tensor_add(res_sb[:, n, :], res_sb[:, n, :], ps)
            nc.vector.tensor_add(res_sb[:, n, :], res_sb[:, n, :], b2_rep)
            nc.sync.dma_start(out=out.rearrange("(n p) e -> p n e", p=128)[:, n, :],
                              in_=res_sb[:, n, :])

    for p in reversed(ctx_pools):
        p.__exit__(None, None, None)


_CACHED = {}


def _get_module(repeat=1, loop_n=None):
    key = ("nc", repeat, loop_n)
    if key not in _CACHED:
        nc = bacc.Bacc("TRN2", target_bir_lowering=False, debug=False,
                       enable_asserts=False, num_devices=NCORES)
        io = declare_io(nc)
        with tile.TileContext(nc) as tc:
            if loop_n is not None:
                with tc.For_i(0, loop_n, 1):
                    build_kernel_body(tc, io, skip_collectives=True)
            else:
                for _ in range(repeat):
                    build_kernel_body(tc, io)
        nc.compile()
        _CACHED[key] = nc
    return _CACHED[key]


def make_in_maps(x, wq, wk, wv, wo, bo, w1, b1, w2, b2, gamma, beta):
    import ml_dtypes
    f = np.float32
    fp = ml_dtypes.bfloat16
    fa = ml_dtypes.bfloat16
    f8 = mybir.dt.np(mybir.dt.float8e4)
    xf = np.asarray(x, f).reshape(B, S, E)
    gamma_f = np.asarray(gamma, f)
    beta_f = np.asarray(beta, f)
    w1_f = np.asarray(w1, f)
    w1g = w1_f * gamma_f[None, :]                 # LN2 gamma folded into w1
    b1g = np.asarray(b1, f) + w1_f @ beta_f       # LN2 beta folded into b1
    common = {
        "wqT": np.ascontiguousarray(np.asarray(wq, f).T.astype(f8)),
        "wkT": np.ascontiguousarray(np.asarray(wk, f).T.astype(f8)),
        "wvT": np.ascontiguousarray(np.asarray(wv, f).T.astype(f8)),
        "woT": np.ascontiguousarray(np.asarray(wo, f).T.astype(f8)),
        "w1T": np.ascontiguousarray(w1g.T.astype(fp)),
        "w2T": np.ascontiguousarray(np.asarray(w2, f).T.astype(fp)),
        "b1rs": np.ascontiguousarray(b1g.reshape(FK, 128).T),
        "bo_row": np.asarray(bo, f).reshape(1, E),
        "b2_row": np.asarray(b2, f).reshape(1, E),
        "g_col": np.ascontiguousarray(gamma_f.reshape(EK, 128).T),
        "be_col": np.ascontiguousarray(beta_f.reshape(EK, 128).T),
    }
    # key offset within a 128-block for mask partition pi = r*32 + jj is
    # r + 4*jj; band mask i (key block 4m+i vs q tile m):
    # visible iff 128*i + keyoff <= g + 4*j
    keyoff = (np.repeat(np.arange(GROUP), 32) +
              4 * np.tile(np.arange(32), GROUP))[:, None]     # [128, 1]
    qq = 4 * np.arange(128)[None, :]                          # [1, 128]
    in_maps = []
    for c in range(NCORES):
        b, g = c // GROUP, c % GROUP
        m = dict(common)
        m["x_own"] = np.ascontiguousarray(xf[b, g::GROUP])
        msk = np.zeros((4, 128, 128), np.float32)
        for i in range(4):
            msk[i] = np.where(128 * i + keyoff <= g + qq, 0.0, NEG)
        m["masks"] = msk.astype(fa)
        in_maps.append(m)
    return in_maps


def run(in_maps, trace=False):
    nc = _get_module()
    return run_bass_kernel_spmd(nc, in_maps, core_ids=list(range(NCORES)),
                                trace=trace)


def kernel(x, wq, wk, wv, wo, bo, w1, b1, w2, b2, gamma, beta):
    in_maps = make_in_maps(x, wq, wk, wv, wo, bo, w1, b1, w2, b2, gamma, beta)
    res = run(in_maps)
    full = np.zeros((B, S, E), np.float32)
    for c in range(NCORES):
        b, g = c // GROUP, c % GROUP
        full[b, g::GROUP] = res.results[c]["out"]
    return full


# revision 28
# speedup vs baseline: 2.1367x; 2.1367x over previous
"""Trainium2 Bass kernel for a dense transformer block (pre-LN MHA + FFN).

Reference computation (B=2, S=2048, E=768, H=12, D=64, FF=3072):
    res = x
    xn = LN(x, gamma, beta)
    q,k,v = xn @ wq.T, xn @ wk.T, xn @ wv.T          (per-head D=64)
    attn = causal_softmax(q k^T / sqrt(D)) v
    res = res + attn @ wo.T + bo
    y = LN(res, gamma, beta)
    out = res + gelu(y @ w1.T + b1) @ w2.T + b2

Sharding: 8 cores, token-parallel with STRIDED assignment for causal load
balance. Core c (group g = c%4, batch b = c//4) owns tokens g::4 of its
batch, so its local q-tile n (128 tokens) covers sequence span
[512n, 512n+512) and needs exactly the key prefix [0, 512(n+1)) — the
same structure on every core. K^T and V are AllGather'ed within each
batch's 4-core group; attention computes only the causal prefix blocks.
The diagonal band (one 128-col region per key block) is masked by
ACCUMULATING -400 into the score PSUM via a matmul with the identity as
stationary (mask values are a per-core input), so exp sees masked scores
with no extra vector-engine op on the critical path.

Layout notes:
  - activations are token-major [tokens(part), feat(free)]; matmuls use
    PE-transposed activation tiles as the stationary operand and
    host-pre-transposed weights as the moving operand.
  - q/k are produced feature-major ([64, T] per head); scores come out
    [keys, q] (keys on partitions); the softmax denominator comes from an
    all-ones column appended to V ("V_aug"); normalization multiplies the
    feature-major attention output via a rank-1 broadcast matmul.
  - LN transposes batch 4 token-tiles into one PSUM bank, then one
    vector copy moves [128, 512] to SBUF.
"""

import os

import numpy as np

import concourse.bass as bass
import concourse.mybir as mybir
import concourse.tile as tile
from concourse import bacc
from concourse.bass_utils import run_bass_kernel_spmd
from concourse.masks import make_identity

F32 = mybir.dt.float32
BF16 = mybir.dt.bfloat16
AF = mybir.ActivationFunctionType

B, S, E, H, FF = 2, 2048, 768, 12, 3072
D = E // H                      # 64
NCORES = 8
T = B * S // NCORES             # 512 tokens per core
TN = T // 128                   # 4 token tiles per core
EK = E // 128                   # 6 feature chunks
FK = FF // 128                  # 24 hidden chunks
GROUP = NCORES // B             # 4 cores per batch
SB = S // 128                   # 16 key blocks per batch
EPS = 1e-5
SCALE = 1.0 / np.sqrt(D)
NEG = -400.0                    # additive mask (pre-scale); exp(-50) ~ 2e-22

DT_A = BF16                     # attention operand dtype
DT_P = BF16                     # FFN operand dtype
DT_X = mybir.dt.float8e4        # QKV/out-proj operand dtype (DoubleRow)
DR = mybir.MatmulPerfMode.DoubleRow
DEBUG_OUTPUTS = bool(int(os.environ.get("KDEBUG", "0")))
KPHASES = os.environ.get("KPHASES", "full")


def _ln(nc, pools, x_ap, out_ap, eps_sb):
    """Normalize (x - mu) * rstd over the free axis (768) of a [128, 768]
    tile. gamma/beta are applied downstream (post-transpose per-partition
    scalars for LN1; folded into w1/b1 for LN2)."""
    stats = pools["stats"].tile([128, 3, 6], F32, tag="stats")
    mv = pools["stats"].tile([128, 2], F32, tag="mv")
    for g in range(3):
        nc.vector.bn_stats(out=stats[:, g, :], in_=x_ap[:, g * 256:(g + 1) * 256])
    nc.vector.bn_aggr(out=mv, in_=stats)
    rstd = pools["stats"].tile([128, 1], F32, tag="rstd")
    nc.scalar.activation(out=rstd, in_=mv[:, 1:2], func=AF.Sqrt, bias=eps_sb, scale=1.0)
    nc.vector.reciprocal(out=rstd, in_=rstd)
    nc.vector.tensor_scalar(
        out=out_ap, in0=x_ap, scalar1=mv[:, 0:1], scalar2=rstd,
        op0=mybir.AluOpType.subtract, op1=mybir.AluOpType.mult,
    )


def declare_io(nc):
    io = {}
    io["x_own"] = nc.dram_tensor("x_own", [T, E], F32, kind="ExternalInput").ap()
    for nm in ("wqT", "wkT", "wvT", "woT"):
        io[nm] = nc.dram_tensor(nm, [E, E], DT_X, kind="ExternalInput").ap()
    io["w1T"] = nc.dram_tensor("w1T", [E, FF], DT_P, kind="ExternalInput").ap()
    io["w2T"] = nc.dram_tensor("w2T", [FF, E], DT_P, kind="ExternalInput").ap()
    io["b1rs"] = nc.dram_tensor("b1rs", [128, FK], F32, kind="ExternalInput").ap()
    io["bo_row"] = nc.dram_tensor("bo_row", [1, E], F32, kind="ExternalInput").ap()
    io["b2_row"] = nc.dram_tensor("b2_row", [1, E], F32, kind="ExternalInput").ap()
    io["g_col"] = nc.dram_tensor("g_col", [128, EK], F32, kind="ExternalInput").ap()
    io["be_col"] = nc.dram_tensor("be_col", [128, EK], F32, kind="ExternalInput").ap()
    io["masks"] = nc.dram_tensor("masks", [4, 128, 128], DT_A, kind="ExternalInput").ap()
    io["out"] = nc.dram_tensor("out", [T, E], F32, kind="ExternalOutput").ap()
    if DEBUG_OUTPUTS:
        for nm, shp, dt in (("dbg_xnT", [128, EK * T], DT_P),
                            ("dbg_qT", [128, EK * T], DT_A),
                            ("dbg_attnT", [128, EK * T], DT_P),
                            ("dbg_res", [128, TN * E], F32)):
            io[nm] = nc.dram_tensor(nm, shp, dt, kind="ExternalOutput").ap()
    return io


def build_kernel_body(tc, io, skip_collectives=False):
    nc = tc.nc
    x_own, wqT, wkT, wvT, woT = (io[k] for k in ("x_own", "wqT", "wkT", "wvT", "woT"))
    w1T, w2T, b1rs = io["w1T"], io["w2T"], io["b1rs"]
    bo_row, b2_row, g_col, be_col = (io[k] for k in ("bo_row", "b2_row", "g_col", "be_col"))
    masks, out = io["masks"], io["out"]

    pools = {}
    ctx_pools = []

    def open_pool(name, **kw):
        cm = tc.tile_pool(name=name, **kw)
        pool = cm.__enter__()
        ctx_pools.append(cm)
        pool._cm = cm
        return pool

    persist = open_pool("persist", bufs=1)
    pools["stats"] = open_pool("stats", bufs=3)
    dram = open_pool("dram", bufs=1, space="DRAM")

    # ---- constants ----
    identity = persist.tile([128, 128], DT_P)
    make_identity(nc, identity)

    ones_all = persist.tile([128, 128], F32)
    nc.vector.memset(ones_all, 1.0)

    eps_sb = persist.tile([128, 1], F32)
    nc.vector.memset(eps_sb, EPS)

    def rep128(name, row_ap):
        t = persist.tile([128, E], F32, name=name)
        src = bass.AP(tensor=row_ap.tensor, offset=row_ap.offset,
                      ap=[[0, 128]] + list(row_ap.ap[1:]))
        nc.sync.dma_start(out=t, in_=src)
        return t

    bo_rep = rep128("bo_rep", bo_row)
    b2_rep = rep128("b2_rep", b2_row)
    b1_sb = persist.tile([128, FK], F32)
    nc.sync.dma_start(out=b1_sb, in_=b1rs)
    g_sb = persist.tile([128, EK], F32)
    nc.sync.dma_start(out=g_sb, in_=g_col)
    be_sb = persist.tile([128, EK], F32)
    nc.sync.dma_start(out=be_sb, in_=be_col)
    mask_sb = persist.tile([128, 4, 128], DT_A)
    nc.sync.dma_start(out=mask_sb, in_=masks.rearrange("t p q -> p t q"))

    # ---- long-lived activations ----
    qT_sb = persist.tile([128, EK, T], DT_A)      # q, feature-major
    attnT_sb = persist.tile([128, EK, T], DT_X)   # attention out, feature-major
    res_sb = persist.tile([128, TN, E], F32)      # post-attn residual, token-major

    # ---- AllGather bounce buffers ----
    VC = H * (D + 1)            # 780: V columns with interleaved ones
    ag_k_in = dram.tile([E, T], DT_A)
    ag_v_in = dram.tile([T, VC], DT_A)
    ag_k_out = dram.tile([GROUP * E, T], DT_A)
    ag_v_out = dram.tile([S, VC], DT_A)
    groups = [list(range(GROUP)), list(range(GROUP, NCORES))]

    # ================= Phase A/B: LN1, x^T, QKV =================
    with tc.tile_pool(name="span_ab", bufs=1) as pab, \
         tc.tile_pool(name="tr_ab", bufs=2) as ptr, \
         tc.tile_pool(name="wproj", bufs=2) as pw, \
         tc.tile_pool(name="psum_ta", bufs=1, space="PSUM") as ppta, \
         tc.tile_pool(name="psum_ab", bufs=3, space="PSUM") as pps:
        x_sb = pab.tile([128, TN, E], F32)
        x_sb_keep = x_sb
        xr = x_own.rearrange("(n p) e -> p n e", p=128)
        for n in range(TN):
            nc.sync.dma_start(out=x_sb[:, n, :], in_=xr[:, n, :])
        xn_b = pab.tile([128, TN, E], DT_P)
        for n in range(TN):
            _ln(nc, pools, x_sb[:, n, :], xn_b[:, n, :], eps_sb)
        xnT_sb = pab.tile([128, EK, T], DT_X)
        for e in range(EK):
            tp = ppta.tile([128, T], DT_P, tag="tp")
            for n in range(TN):
                nc.tensor.transpose(tp[:, n * 128:(n + 1) * 128],
                                    xn_b[:, n, e * 128:(e + 1) * 128], identity)
            nc.vector.tensor_scalar(
                out=xnT_sb[:, e, :], in0=tp, scalar1=g_sb[:, e:e + 1],
                scalar2=be_sb[:, e:e + 1],
                op0=mybir.AluOpType.mult, op1=mybir.AluOpType.add)

        # k^T first (its AllGather gates nothing downstream of qT), then V
        # (its AllGather gates attention head 0), then local q^T last.
        wk_sb = pw.tile([128, EK, E], DT_X, tag="w")
        wkr = wkT.rearrange("(k p) f -> p k f", p=128)
        for k in range(EK):
            nc.sync.dma_start(out=wk_sb[:, k, :], in_=wkr[:, k, :])
        for m in range(EK):
            ps = pps.tile([128, T], F32, tag="mm")
            for c in range(EK // 2):
                nc.tensor.matmul(ps, wk_sb[:, 2 * c:2 * c + 2, m * 128:(m + 1) * 128],
                                 xnT_sb[:, 2 * c:2 * c + 2, :],
                                 start=(c == 0), stop=(c == EK // 2 - 1), perf_mode=DR)
            kcp = ptr.tile([128, T], DT_A, tag="kcp")
            nc.vector.tensor_copy(kcp, ps)
            nc.sync.dma_start(out=ag_k_in[m * 128:(m + 1) * 128, :], in_=kcp)

        # V: token-major [T, 768]
        wv_sb = pw.tile([128, EK, E], DT_X, tag="w")
        wvr = wvT.rearrange("(k p) f -> p k f", p=128)
        for k in range(EK):
            nc.sync.dma_start(out=wv_sb[:, k, :], in_=wvr[:, k, :])
        for n in range(TN):
            psv = pps.tile([128, E], F32, tag="mmv", bufs=2)
            for c in range(EK // 2):
                lhsT = xnT_sb[:, 2 * c:2 * c + 2, n * 128:(n + 1) * 128]
                nc.tensor.matmul(psv[:, 0:512], lhsT, wv_sb[:, 2 * c:2 * c + 2, 0:512],
                                 start=(c == 0), stop=(c == EK // 2 - 1), perf_mode=DR)
                nc.tensor.matmul(psv[:, 512:768], lhsT,
                                 wv_sb[:, 2 * c:2 * c + 2, 512:768],
                                 start=(c == 0), stop=(c == EK // 2 - 1), perf_mode=DR)
            vcp = ptr.tile([128, VC], DT_A, tag="vcp")
            vcpr = vcp.rearrange("p (h c) -> p h c", h=H, c=D + 1)
            nc.vector.tensor_copy(vcpr[:, :, 0:D],
                                  psv.rearrange("p (h c) -> p h c", h=H, c=D))
            nc.vector.memset(vcpr[:, :, D:D + 1], 1.0)
            nc.sync.dma_start(
                out=ag_v_in.rearrange("(n p) c -> p n c", p=128)[:, n, :], in_=vcp)

        # q^T: local only, overlaps the in-flight AllGathers
        wq_sb = pw.tile([128, EK, E], DT_X, tag="w")
        wqr = wqT.rearrange("(k p) f -> p k f", p=128)
        for k in range(EK):
            nc.sync.dma_start(out=wq_sb[:, k, :], in_=wqr[:, k, :])
        for m in range(EK):
            ps = pps.tile([128, T], F32, tag="mm")
            for c in range(EK // 2):
                nc.tensor.matmul(ps, wq_sb[:, 2 * c:2 * c + 2, m * 128:(m + 1) * 128],
                                 xnT_sb[:, 2 * c:2 * c + 2, :],
                                 start=(c == 0), stop=(c == EK // 2 - 1), perf_mode=DR)
            nc.vector.tensor_copy(qT_sb[:, m, :], ps)


    if DEBUG_OUTPUTS:
        nc.sync.dma_start(out=io["dbg_xnT"],
                          in_=xnT_sb.rearrange("p k t -> p (k t)"))
        nc.sync.dma_start(out=io["dbg_qT"], in_=qT_sb.rearrange("p k t -> p (k t)"))

    # ================= Phase C: AllGather K^T and V =================
    if not skip_collectives:
        nc.gpsimd.collective_compute("AllGather", mybir.AluOpType.bypass,
                                     replica_groups=groups,
                                     ins=[ag_k_in[:]], outs=[ag_k_out[:]])
        nc.gpsimd.collective_compute("AllGather", mybir.AluOpType.bypass,
                                     replica_groups=groups,
                                     ins=[ag_v_in[:]], outs=[ag_v_out[:]])

    # ---- open late-phase pools early so weight DMAs overlap attention ----
    span_fgh = open_pool("span_fgh", bufs=1)
    hT_sb = span_fgh.tile([128, FK, T], DT_P)     # FFN hidden, feature-major
    wo_sb = span_fgh.tile([128, EK, E], DT_X)
    wor = woT.rearrange("(k p) f -> p k f", p=128)
    for k in range(EK):
        nc.gpsimd.dma_start(out=wo_sb[:, k, :], in_=wor[:, k, :])
    w1_sb = span_fgh.tile([128, EK, FF], DT_P)
    w1r = w1T.rearrange("(k p) f -> p k f", p=128)
    for k in range(EK):
        for j in range(2):
            nc.gpsimd.dma_start(out=w1_sb[:, k, j * 1536:(j + 1) * 1536],
                                in_=w1r[:, k, j * 1536:(j + 1) * 1536])
    w2_sb = span_fgh.tile([128, FK, E], DT_P)
    w2r = w2T.rearrange("(k p) f -> p k f", p=128)
    for k in range(0, FK, 2):
        nc.gpsimd.dma_start(out=w2_sb[:, k:k + 2, :], in_=w2r[:, k:k + 2, :])
    x_rl = span_fgh.tile([128, TN, E], F32)
    xr2 = x_own.rearrange("(n p) e -> p n e", p=128)
    for n in range(TN):
        nc.gpsimd.dma_start(out=x_rl[:, n, :], in_=xr2[:, n, :])
    for n in range(TN):
        nc.gpsimd.tensor_add(x_rl[:, n, :], x_rl[:, n, :], bo_rep)

    if KPHASES == "ab":
        outr0 = out.rearrange("(n p) e -> p n e", p=128)
        nc.sync.dma_start(out=outr0[:, 0, :], in_=x_sb_keep[:, 0, :])
        for p in reversed(ctx_pools):
            p.__exit__(None, None, None)
        return

    # ================= Phase D/E: attention =================
    # Per head: 8 pairs of key blocks, pair pi=(m,p) holds blocks t=4m+2p+j
    # (j=0,1), each computing scores for the q suffix [128m, 512). The local
    # first 128 columns of each block's region are its diagonal band: add
    # NEG there via matmul (identity stationary, mask moving), then exp.
    with tc.tile_pool(name="attn_v", bufs=1) as pvb, \
         tc.tile_pool(name="attn_kv", bufs=2) as pkv, \
         tc.tile_pool(name="attn_exp", bufs=4) as pexp, \
         tc.tile_pool(name="attn_nrm", bufs=2) as pnrm, \
         tc.tile_pool(name="psum_s", bufs=3, space="PSUM") as pps_s, \
         tc.tile_pool(name="psum_a", bufs=2, space="PSUM") as pps_a:
        kgr = ag_k_out.rearrange("(r hp p) t -> p r hp t", r=GROUP, hp=H // 2, p=2 * D)
        vgr = ag_v_out.rearrange("(r n p) c -> r p n c", r=GROUP, n=SB, p=32)
        vbig = pvb.tile([128, SB, VC], DT_A)
        for r in range(GROUP):
            nc.sync.dma_start(out=vbig[r * 32:(r + 1) * 32, :, :], in_=vgr[r])
        run_attn = KPHASES != "abfgh"
        if not run_attn:
            nc.vector.memset(attnT_sb[:, 0, 0:1], 0.0)

        def emit_qk(st):
            ps_pair, m, p, h = st["ps"], st["m"], st["p"], st["h"]
            khead = st["khead"]
            base = (h % 2) * D
            L = T - 128 * m
            qs = 128 * m
            for j in range(2):
                t = 4 * m + 2 * p + j
                kap = khead[base:base + D, t, :]          # 128 keys, contiguous
                nc.tensor.matmul(ps_pair[:, j, 0:L], kap,
                                 qT_sb[base:base + D, h // 2, qs:T],
                                 start=True, stop=False)
            for j in range(2):
                i = 2 * p + j
                nc.tensor.matmul(ps_pair[:, j, 0:128], identity, mask_sb[:, i, :],
                                 start=False, stop=True)

        def emit_exp(st):
            ps_pair, m = st["ps"], st["m"]
            L = T - 128 * m
            ex = pexp.tile([128, 2, T], DT_A, tag="ex")
            st["ex"] = ex
            nc.scalar.activation(out=ex[:, :, 0:L], in_=ps_pair[:, :, 0:L],
                                 func=AF.Exp, scale=SCALE)

        def emit_pv(st):
            ex, ps_attn, m, p, h = st["ex"], st["ps_attn"], st["m"], st["p"], st["h"]
            L = T - 128 * m
            qs = 128 * m
            for j in range(2):
                t = 4 * m + 2 * p + j
                nc.tensor.matmul(ps_attn[0:D + 1, qs:T],
                                 vbig[:, t, h * (D + 1):(h + 1) * (D + 1)],
                                 ex[:, j, 0:L], start=(t == 0), stop=(t == SB - 1))

        def emit_norm(hst):
            ps_attn, h = hst["ps_attn"], hst["h"]
            att = pnrm.tile([D + 1, T], F32, tag="att")
            nc.vector.tensor_copy(att, ps_attn[0:D + 1, :])
            nc.vector.reciprocal(att[D:D + 1, :], att[D:D + 1, :])
            ps_r = pps_s.tile([128, 2, T], F32, tag="ps")
            nc.tensor.matmul(ps_r[0:D, 0, :], ones_all[D:D + 1, 0:D],
                             att[D:D + 1, :], start=True, stop=True)
            nc.vector.tensor_mul(
                attnT_sb[(h % 2) * D:(h % 2 + 1) * D, h // 2, :],
                att[0:D, :], ps_r[0:D, 0, :])

        prev_head = None
        khead = None
        for h in range(H if run_attn else 0):
            if h % 2 == 0:
                # khead[:, t, r*32+jj] = K^T col (token r + 4*(32t+jj));
                # blocks of 128 keys contiguous per t (stationary needs 1D free)
                khead = pkv.tile([128, SB, 128], DT_A, tag="khead")
                for r in range(GROUP):
                    nc.sync.dma_start(
                        out=khead[:, :, r * 32:(r + 1) * 32],
                        in_=kgr[:, r, h // 2, :].rearrange("p (t j) -> p t j", j=32))
            ps_attn = pps_a.tile([D + 1, T], F32, tag="pattn")
            hst = {"h": h, "ps_attn": ps_attn}

            # software pipeline: QK(0) QK(1) exp(0) QK(2) exp(1) PV(0)
            #                    QK(3) exp(2) PV(1) ... exp(7) PV(6) PV(7)
            sts = []
            for pi in range(8):
                ps_pair = pps_s.tile([128, 2, T], F32, tag="ps", name="ps_pair")
                st = {"m": pi // 2, "p": pi % 2, "khead": khead,
                      "h": h, "ps_attn": ps_attn, "ps": ps_pair}
                sts.append(st)
                emit_qk(st)
                if pi >= 1:
                    emit_exp(sts[pi - 1])
                if pi == 2 and prev_head is not None:
                    emit_norm(prev_head)
                if pi >= 2:
                    emit_pv(sts[pi - 2])
            emit_exp(sts[7])
            emit_pv(sts[6])
            emit_pv(sts[7])
            prev_head = hst
        if run_attn:
            emit_norm(prev_head)

    if DEBUG_OUTPUTS:
        nc.sync.dma_start(out=io["dbg_attnT"],
                          in_=attnT_sb.rearrange("p k t -> p (k t)"))

    # ================= Phases F-H =================
    if KPHASES == "abd":
        outr0 = out.rearrange("(n p) e -> p n e", p=128)
        nc.sync.dma_start(out=outr0[:, 0, :], in_=x_sb_keep[:, 0, :])
        for p in reversed(ctx_pools):
            p.__exit__(None, None, None)
        return

    span_fg = open_pool("span_fg", bufs=1)
    y2n_b = span_fg.tile([128, TN, E], DT_P)
    y2nT_sb = span_fg.tile([128, EK, T], DT_P)    # LN2 output, feature-major

    # ---- Phase F: out-proj + residual + LN2 ----
    with tc.tile_pool(name="psum_tf", bufs=1, space="PSUM") as pptf, \
         tc.tile_pool(name="psum_f", bufs=2, space="PSUM") as ppsf:
        tpf = pptf.tile([128, EK, T], DT_P)
        for n in range(TN):
            psf = ppsf.tile([128, E], F32, tag="mm")
            for c in range(EK // 2):
                lhsT = attnT_sb[:, 2 * c:2 * c + 2, n * 128:(n + 1) * 128]
                nc.tensor.matmul(psf[:, 0:512], lhsT, wo_sb[:, 2 * c:2 * c + 2, 0:512],
                                 start=(c == 0), stop=(c == EK // 2 - 1), perf_mode=DR)
                nc.tensor.matmul(psf[:, 512:768], lhsT,
                                 wo_sb[:, 2 * c:2 * c + 2, 512:768],
                                 start=(c == 0), stop=(c == EK // 2 - 1), perf_mode=DR)
            nc.vector.tensor_add(res_sb[:, n, :], psf, x_rl[:, n, :])
            _ln(nc, pools, res_sb[:, n, :], y2n_b[:, n, :], eps_sb)
            for e in range(EK):
                nc.tensor.transpose(tpf[:, e, n * 128:(n + 1) * 128],
                                    y2n_b[:, n, e * 128:(e + 1) * 128], identity)
        for e in range(EK):
            nc.vector.tensor_copy(y2nT_sb[:, e, :], tpf[:, e, :])

    if DEBUG_OUTPUTS:
        nc.sync.dma_start(out=io["dbg_res"],
                          in_=res_sb.rearrange("p n e -> p (n e)"))

    # ================= Phase G: FFN1 + GELU -> hT =================
    with tc.tile_pool(name="psum_g", bufs=4, space="PSUM") as ppsg:
        for m in range(FK):
            ps = ppsg.tile([128, T], F32, tag="mm")
            for k in range(EK):
                nc.tensor.matmul(ps, w1_sb[:, k, m * 128:(m + 1) * 128],
                                 y2nT_sb[:, k, :], start=(k == 0), stop=(k == EK - 1))
            nc.scalar.activation(out=hT_sb[:, m, :], in_=ps, func=AF.Gelu,
                                 bias=b1_sb[:, m:m + 1], scale=1.0)

    # close span_fg (y2nT dead) before loading anything else
    ctx_pools.remove(span_fg._cm)
    span_fg._cm.__exit__(None, None, None)

    # ================= Phase H: FFN2 + residual -> out =================
    with tc.tile_pool(name="psum_h", bufs=3, space="PSUM") as ppsh:
        for n in range(TN):
            ps = ppsh.tile([128, E], F32, tag="mm")
            for k in range(FK):
                lhsT = hT_sb[:, k, n * 128:(n + 1) * 128]
                nc.tensor.matmul(ps[:, 0:512], lhsT, w2_sb[:, k, 0:512],
                                 start=(k == 0), stop=(k == FK - 1))
                nc.tensor.matmul(ps[:, 512:768], lhsT, w2_sb[:, k, 512:768],
                                 start=(k == 0), stop=(k == FK - 1))
            nc.vector.tensor_add(res_sb[:, n, :], res_sb[:, n, :], ps)
            nc.vector.tensor_add(res_sb[:, n, :], res_sb[:, n, :], b2_rep)
            nc.sync.dma_start(out=out.rearrange("(n p) e -> p n e", p=128)[:, n, :],
                              in_=res_sb[:, n, :])

    for p in reversed(ctx_pools):
        p.__exit__(None, None, None)


_CACHED = {}


def _get_module(repeat=1, loop_n=None):
    key = ("nc", repeat, loop_n)
    if key not in _CACHED:
        nc = bacc.Bacc("TRN2", target_bir_lowering=False, debug=False,
                       enable_asserts=False, num_devices=NCORES)
        io = declare_io(nc)
        with tile.TileContext(nc) as tc:
            if loop_n is not None:
                with tc.For_i(0, loop_n, 1):
                    build_kernel_body(tc, io, skip_collectives=True)
            else:
                for _ in range(repeat):
                    build_kernel_body(tc, io)
        nc.compile()
        _CACHED[key] = nc
    return _CACHED[key]


def make_in_maps(x, wq, wk, wv, wo, bo, w1, b1, w2, b2, gamma, beta):
    import ml_dtypes
    f = np.float32
    fp = ml_dtypes.bfloat16
    fa = ml_dtypes.bfloat16
    f8 = mybir.dt.np(mybir.dt.float8e4)
    xf = np.asarray(x, f).reshape(B, S, E)
    gamma_f = np.asarray(gamma, f)
    beta_f = np.asarray(beta, f)
    w1_f = np.asarray(w1, f)
    w1g = w1_f * gamma_f[None, :]                 # LN2 gamma folded into w1
    b1g = np.asarray(b1, f) + w1_f @ beta_f       # LN2 beta folded into b1
    common = {
        "wqT": np.ascontiguousarray(np.asarray(wq, f).T.astype(f8)),
        "wkT": np.ascontiguousarray(np.asarray(wk, f).T.astype(f8)),
        "wvT": np.ascontiguousarray(np.asarray(wv, f).T.astype(f8)),
        "woT": np.ascontiguousarray(np.asarray(wo, f).T.astype(f8)),
        "w1T": np.ascontiguousarray(w1g.T.astype(fp)),
        "w2T": np.ascontiguousarray(np.asarray(w2, f).T.astype(fp)),
        "b1rs": np.ascontiguousarray(b1g.reshape(FK, 128).T),
        "bo_row": np.asarray(bo, f).reshape(1, E),
        "b2_row": np.asarray(b2, f).reshape(1, E),
        "g_col": np.ascontiguousarray(gamma_f.reshape(EK, 128).T),
        "be_col": np.ascontiguousarray(beta_f.reshape(EK, 128).T),
    }
    # key offset within a 128-block for mask partition pi = r*32 + jj is
    # r + 4*jj; band mask i (key block 4m+i vs q tile m):
    # visible iff 128*i + keyoff <= g + 4*j
    keyoff = (np.repeat(np.arange(GROUP), 32) +
              4 * np.tile(np.arange(32), GROUP))[:, None]     # [128, 1]
    qq = 4 * np.arange(128)[None, :]                          # [1, 128]
    in_maps = []
    for c in range(NCORES):
        b, g = c // GROUP, c % GROUP
        m = dict(common)
        m["x_own"] = np.ascontiguousarray(xf[b, g::GROUP])
        msk = np.zeros((4, 128, 128), np.float32)
        for i in range(4):
            msk[i] = np.where(128 * i + keyoff <= g + qq, 0.0, NEG)
        m["masks"] = msk.astype(fa)
        in_maps.append(m)
    return in_maps


def run(in_maps, trace=False):
    nc = _get_module()
    return run_bass_kernel_spmd(nc, in_maps, core_ids=list(range(NCORES)),
                                trace=trace)


def kernel(x, wq, wk, wv, wo, bo, w1, b1, w2, b2, gamma, beta):
    in_maps = make_in_maps(x, wq, wk, wv, wo, bo, w1, b1, w2, b2, gamma, beta)
    res = run(in_maps)
    full = np.zeros((B, S, E), np.float32)
    for c in range(NCORES):
        b, g = c // GROUP, c % GROUP
        full[b, g::GROUP] = res.results[c]["out"]
    return full
